# revision 30
# baseline (speedup 1.0000x reference)
"""MeanStdFilter kernel for 8 Trainium2 NeuronCores.

Semantics (matches the sequential-Welford reference with M=0, S=S_in, n=0):
    S1[f] = sum_b x[b, f]            (global, over all 32768 rows)
    S2[f] = sum_b x[b, f]^2
    mean  = S1 / N
    M2    = S2 - S1^2 / N + S_in     (Welford M2 started from buffer S)
    var   = M2 / (N - 1)             (N = 32768 > 1)
    out   = (x - mean) / (sqrt(var) + 1e-5)
The input running-mean buffer M is overwritten by the first Welford step in
the reference, so it never affects the output.

Design (v14). Critical path is the CC stack, NOT the load: engine-init
preamble (~7-10us) + doorbell pickup (~8-14) + per-execution CC
barrier (22-111us, heavy-tailed jitter, median ~41; VERIFIED
per-execution -- a warm prior exec does not remove it) + ~11.2us ncfw
gap + warmup exec (~10.5) + ~1.8 + real AR exec (~10.3). The load
phase (16 MiB fp32 @ ~315-400 GB/s, ends ~52-62us) hides under it in
all but the fastest-barrier runs. Structure:
  - x sharded 4096 rows/core, streamed fp32 (1 MB two-tile DMAs), kept
    resident as BF16 (ACT casts, DVE squares, PE matmuls). lhsT =
    [128, 2] of 1/B (= 2^-15, exact in bf16) -> PSUM accumulates 2
    replicated rows of S1/B, S2/B directly (products are just
    exponent-shifted; no precision loss, no post-AR rescale). The last
    chunk uses per-tile cast/square ops (2D views -- [P,1,F] middle-dim
    sliced writes raced with their matmul readers) so the final
    matmuls start ~1us sooner.
  - Warmup AllReduce on an UNINITIALIZED dram tile (no input deps ->
    gpsimd doorbell fires at engine-init ~7us) absorbs the CC barrier
    + ncfw warmup under the load phase. It MUST be an AllReduce: a
    mismatched warmup op (AllGather, v8) blew the barrier init from
    ~47 to 116us. Tiny payload (a same-size warmup exec costs +2.4us).
  - S_in/(8B) is folded into the pre-AR staging add (each core adds
    1/8), removing the post-AR "+Sin" op.
  - 16KB ncfw AllReduce of the stats, packed INTERLEAVED by feature
    quarter: block q = [S1q | S2q], so each post-AR broadcast DMA
    lands both operands of its quarter-chain together.
  - Post-AR: four 256KB broadcast DMAs (land = ~2.2us descriptor-gen +
    ~1.2 data + ~1.4 receipt after issue; the land cost is mostly
    FIXED, which is why quartering the payload pays). Quarter chain:
    DVE u=m*m -> DVE M2b=s2-u -> ACT rstd=rsqrt(M2b*B/(B-1)) (bf16
    out), with ACT m->bf16 issued at DMA-land so the DVE sub hides
    under rstd. Phase C computes out = (x - m)*rstd, so no -mean*rstd
    product on the critical path; tiles 0 AND 1 quarter-normalize +
    store (8x128KB at ~0.7us cadence) right behind each rstd quarter,
    feeding the store stream while the later quarter DMAs land. First
    store issues ~AR+8.5-9.5us (was +14.9 before the quartering).
  - Phase C normalizes the bf16 copy in place (2x packed DVE sub+mult),
    ACT upcasts per-tile, 512KB stores (~350 GB/s steady).
  NOTE: the CC barrier jitter makes single runs vary -- observed range
  147-232us with identical code; median ~160. Compare configurations
  only via multi-run medians.
"""

import functools

import numpy as np

import concourse.bacc as bacc
import concourse.tile as tile
from concourse import mybir
from concourse.bass_utils import run_bass_kernel_spmd

NCORES = 8
B, F = 32768, 1024
ROWS = B // NCORES  # 4096 rows per core
P = 128
NT = ROWS // P  # 32 row-tiles of [128, 1024] per core
R = 2  # stat replicas (partition rows) carried through the AR
EPS = 1e-5
FP32 = mybir.dt.float32
BF16 = mybir.dt.bfloat16
AF = mybir.ActivationFunctionType
ALU = mybir.AluOpType
INV_B = 1.0 / B  # 2^-15, exact in bf16


def build_kernel():
    nc = bacc.Bacc(
        "TRN2", target_bir_lowering=False, debug=False, num_devices=NCORES
    )
    x = nc.declare_dram_parameter("x", [ROWS, F], FP32, isOutput=False)
    s_in = nc.declare_dram_parameter("S", [1, F], FP32, isOutput=False)
    out = nc.declare_dram_parameter("out", [ROWS, F], FP32, isOutput=True)

    x_t2 = x[:].rearrange("(c n p) f -> c p n f", n=2, p=P)
    out_t = out[:].rearrange("(n p) f -> n p f", p=P)
    groups = [list(range(NCORES))]

    with tile.TileContext(nc) as tc:
        with (
            tc.tile_pool(name="xf", bufs=6) as xfpool,
            tc.tile_pool(name="xb", bufs=1) as xbpool,
            tc.tile_pool(name="sq", bufs=3) as sqpool,
            tc.tile_pool(name="o32", bufs=8) as opool,
            tc.tile_pool(name="stats", bufs=1) as stats,
            tc.tile_pool(name="psum", bufs=1, space="PSUM") as psum,
            tc.tile_pool(name="dram", bufs=1, space="DRAM") as dram,
        ):
            # Resident bf16 shard: 4 chunks x [128, 8, 1024] (16 KB/part each).
            xb = [
                xbpool.tile([P, 8, F], BF16, tag=f"xb{c}", name=f"xb{c}")
                for c in range(4)
            ]

            def xtile(t):
                return xb[t // 8][:, t % 8, :]

            # lhsT for the stats matmuls: [128, R] of 1/B (= 2^-15, exact in
            # bf16) -> PSUM accumulates R identical rows of S1/B, S2/B.
            onesB = stats.tile([P, R], BF16)
            nc.vector.memset(onesB, INV_B)

            # Warmup AllReduce on an uninitialized dram tile: zero input
            # dependencies, so the gpsimd doorbell fires at engine-init and
            # the one-time CC barrier + warmup exec complete under the load
            # phase, leaving the CC stream free for the real AR. Same OP as
            # the real collective: v8 showed a mismatched warmup (AllGather)
            # blows the barrier init from ~47us to 116us. Tiny payload: the
            # warmup exec itself is ~2.4us cheaper than a same-size one.
            wu_in = dram.tile([1, 8], FP32)
            wu_out = dram.tile([1, 8], FP32)
            nc.gpsimd.collective_compute(
                "AllReduce",
                ALU.add,
                replica_groups=groups,
                ins=[wu_in[:].opt()],
                outs=[wu_out[:].opt()],
            )

            # One PSUM bank per 512-wide half (4 banks total).
            ps1 = [psum.tile([R, 512], FP32, tag=f"ps1_{h}", name=f"ps1_{h}") for h in range(2)]
            ps2 = [psum.tile([R, 512], FP32, tag=f"ps2_{h}", name=f"ps2_{h}") for h in range(2)]

            # ---- Phase A: stream fp32 tiles (1 MB two-tile DMAs halve the
            # per-DMA fixed costs), cast to bf16, square, PE sums.
            prewarm = stats.tile([P, 8], FP32)
            sin_row = stats.tile([R, F], FP32)
            sin8b = stats.tile([R, F], FP32)
            for c in range(NT // 2):
                t0, t1 = 2 * c, 2 * c + 1
                xf2 = xfpool.tile([P, 2, F], FP32, tag="xf")
                nc.sync.dma_start(out=xf2[:], in_=x_t2[c])
                # Two-tile compute ops (FD=2048) halve per-op DRAIN/sem
                # overhead on ACT and DVE -- except the LAST chunk, which
                # uses per-tile ops so tile 30's matmuls start ~1us sooner
                # (shorter load-end -> AR-doorbell tail).
                xb2 = xb[t0 // 8][:, t0 % 8 : t0 % 8 + 2, :]
                sq = sqpool.tile([P, 2, F], BF16, tag="sq")
                if c == NT // 2 - 1:
                    for k in range(2):
                        nc.scalar.activation(
                            xtile(t0 + k), xf2[:, k, :], AF.Copy
                        )
                        nc.vector.tensor_tensor(
                            sq[:, k, :],
                            xf2[:, k, :],
                            xf2[:, k, :],
                            ALU.mult,
                        )
                else:
                    nc.scalar.activation(xb2, xf2, AF.Copy)  # fp32 -> bf16
                    nc.vector.tensor_tensor(sq[:], xf2, xf2, ALU.mult)  # x^2
                for t in (t0, t1):
                    for h in range(2):
                        hs = slice(h * 512, (h + 1) * 512)
                        nc.tensor.matmul(
                            ps1[h][:],
                            lhsT=onesB[:],
                            rhs=xtile(t)[:, hs],
                            start=(t == 0),
                            stop=(t == NT - 1),
                        )
                        nc.tensor.matmul(
                            ps2[h][:],
                            lhsT=onesB[:],
                            rhs=sq[:, t % 2, hs],
                            start=(t == 0),
                            stop=(t == NT - 1),
                        )
                if c == 0:
                    # Pre-load the ACT rsqrt LUT so finalize doesn't pay the
                    # ~2.7us ACT_TABLE_LOAD on the critical path. Also load
                    # S_in (replicated to R partitions) and pre-scale it by
                    # 1/(8B): each core folds 1/8 of S_in into its staged
                    # S2/B partial before the AR.
                    nc.vector.memset(prewarm, 1.0)
                    nc.scalar.activation(
                        prewarm, prewarm, AF.Abs_reciprocal_sqrt
                    )
                    nc.sync.dma_start(
                        out=sin_row[:], in_=s_in[:].to_broadcast([R, F])
                    )
                    nc.scalar.activation(
                        sin8b, sin_row, AF.Copy, scale=INV_B / NCORES
                    )

            # Pack the stats INTERLEAVED by quarter: block q (512 wide) =
            # [S1q | S2q] for feature quarter q, into one [R, 2048] staging
            # tile for the AR, so ONE post-AR broadcast DMA lands both
            # operands of each quarter-chain together. S1 parts on ACT, S2
            # parts (+Sin fold) on DVE so they drain in parallel.
            Q = F // 4
            cc_stage = stats.tile([R, 2 * F], FP32)
            for h in range(2):
                for j in range(2):
                    q = 2 * h + j
                    nc.scalar.copy(
                        cc_stage[:, q * 2 * Q : q * 2 * Q + Q],
                        ps1[h][:, j * Q : (j + 1) * Q],
                    )
                    nc.vector.tensor_tensor(
                        cc_stage[:, q * 2 * Q + Q : (q + 1) * 2 * Q],
                        ps2[h][:, j * Q : (j + 1) * Q],
                        sin8b[:, q * Q : (q + 1) * Q],
                        ALU.add,
                    )

            cc_in = dram.tile([R, 2 * F], FP32)
            cc_out = dram.tile([R, 2 * F], FP32)
            nc.sync.dma_start(out=cc_in[:], in_=cc_stage[:])
            nc.gpsimd.collective_compute(
                "AllReduce",
                ALU.add,
                replica_groups=groups,
                ins=[cc_in[:].opt()],
                outs=[cc_out[:].opt()],
            )

            # ---- Broadcast-redundant finalize, PIPELINED BY FEATURE
            # QUARTER. Four 256KB broadcast DMAs; DMA q lands the
            # interleaved block [S1q | S2q] (alternating replica rows), so
            # the q-chain (u -> M2b -> rstd -> tile-0 quarter norm ->
            # store) has BOTH operands the moment its single DMA lands
            # (~AR+5.5); the land latency is ~fixed, so quartering shrinks
            # the payload wait without adding serial waits. Every partition
            # computes the per-feature stats redundantly.
            ms = [
                stats.tile([P, 2 * Q], FP32, tag=f"ms{q}", name=f"ms{q}")
                for q in range(4)
            ]
            u_b = stats.tile([P, F], FP32)
            m_b16 = stats.tile([P, F], BF16)
            rstd_b = stats.tile([P, F], BF16)
            for q in range(4):
                nc.sync.dma_start(
                    out=ms[q][:],
                    in_=cc_out[
                        q % R : q % R + 1, q * 2 * Q : (q + 1) * 2 * Q
                    ].to_broadcast([P, 2 * Q]),
                )
            xb0, xb1 = xtile(0), xtile(1)
            # Phase C computes out = (x - m) * rstd, so the finalize needs
            # only m (bf16 cast) and rstd -- no -mean*rstd product, which
            # kept a serial DVE dependency on the critical path.
            # rstd = 1/sqrt(M2/B * B/(B-1)) straight to bf16 on ACT (phase C
            # consumes bf16). The +eps on std shifts the result ~1e-5
            # relative, far below bf16 resolution.
            RSQ_SCALE = float(B) / (B - 1)
            for q in range(4):
                qs = slice(q * Q, (q + 1) * Q)
                m_q = ms[q][:, 0:Q]
                s2_q = ms[q][:, Q : 2 * Q]
                # ACT order: mcast FIRST (needs only the DMA land), so the
                # DVE sub can run under the rstd compute.
                nc.scalar.activation(m_b16[:, qs], m_q, AF.Copy)  # m->bf16
                nc.vector.tensor_tensor(u_b[:, qs], m_q, m_q, ALU.mult)
                nc.vector.tensor_tensor(
                    s2_q, s2_q, u_b[:, qs], ALU.subtract
                )  # M2/B
                nc.scalar.activation(
                    rstd_b[:, qs], s2_q, AF.Abs_reciprocal_sqrt,
                    scale=RSQ_SCALE,
                )
                # Tiles 0 AND 1 quarter-normalize right behind each rstd
                # quarter: their 8x128KB stores feed the store stream while
                # the later quarter DMAs land (the ramp used to trickle at
                # ~170 GB/s because the steady loop waited on all four
                # quarter chains).
                for xbt, ot in ((xb0, out_t[0]), (xb1, out_t[1])):
                    nc.vector.tensor_tensor(
                        xbt[:, qs], xbt[:, qs], m_b16[:, qs], ALU.subtract
                    )
                    nc.vector.tensor_tensor(
                        xbt[:, qs], xbt[:, qs], rstd_b[:, qs], ALU.mult
                    )
                    o32 = opool.tile([P, Q], FP32, tag="o32q")
                    nc.scalar.activation(o32, xbt[:, qs], AF.Copy)
                    nc.sync.dma_start(out=ot[:, qs], in_=o32[:])

            # Stride-0 middle-dim views feed the two-tile phase C ops
            # without materializing duplicate lanes.
            rstd_b2 = rstd_b[:, None, :].to_broadcast([P, 2, F])
            mb2 = m_b16[:, None, :].to_broadcast([P, 2, F])
            for c in range(1, NT // 2):
                t0 = 2 * c
                xb2 = xb[t0 // 8][:, t0 % 8 : t0 % 8 + 2, :]
                nc.vector.tensor_tensor(xb2, xb2, mb2, ALU.subtract)
                nc.vector.tensor_tensor(xb2, xb2, rstd_b2, ALU.mult)
                # Per-tile casts + stores keep the store stream smooth (a
                # single 2-tile cast before paired stores measured ~20 GB/s
                # slower stores).
                for k in range(2):
                    o32 = opool.tile([P, F], FP32, tag="o32")
                    nc.scalar.activation(o32, xb2[:, k, :], AF.Copy)
                    nc.sync.dma_start(out=out_t[t0 + k], in_=o32[:])

    nc.finalize()
    return nc


@functools.cache
def _get_nc():
    return build_kernel()


def kernel(x, M, S, _trace=False, _trace_kwargs=None):
    del M  # overwritten by the first Welford step in the reference
    x = np.ascontiguousarray(x, dtype=np.float32)
    S = np.ascontiguousarray(S, dtype=np.float32).reshape(1, F)
    nc = _get_nc()
    in_maps = [
        {"x": x[i * ROWS : (i + 1) * ROWS], "S": S} for i in range(NCORES)
    ]
    res = run_bass_kernel_spmd(
        nc,
        in_maps,
        core_ids=list(range(NCORES)),
        trace=_trace,
        **(_trace_kwargs or {}),
    )
    out = np.concatenate([res.results[i]["out"] for i in range(NCORES)], axis=0)
    if _trace:
        return out, res
    return out
